# revision 4
# baseline (speedup 1.0000x reference)
"""Trainium2 Bass kernel for nn_CANNLinear (4-bit per-tensor symmetric weight
quantization + dense linear), column-parallel over 8 NeuronCores.

Computation (matches the reference exactly at the quantization step):
    scale  = max(max(|W|) * fl(1/7), 1e-8)        (global over full W, AllReduce max)
    q      = round(W * fl(1/scale))               (RNE round via +/-1.5*2^23)
    out    = x @ (q * scale)^T + bias

fp8 fast path: q in [-7,7] is EXACT in float8e4 (e4m3), and x is split
hi/lo:  x_hi = fp8(x), x_lo = fp8(x - x_hi), so  x_hi + x_lo ~ x to ~2^-8
relative.  Both planes matmul against the same fp8 q with
MatmulPerfMode.DoubleRow (2 k-tiles per instruction, 0.5 cycles/row), so the
2x FLOPs of the two planes run at 2-4x the bf16 rate => net >= bf16x2 speed
with bf16-class accuracy (measured rel err ~7e-4 vs 2e-2 gate).

Transpose trick for 1-byte operands (xbar is 2-byte only): view fp8 pairs
along k as uint16 and xbar-transpose the u16 tensor.  The transposed tile at
partition p, chunk t holds k = 2*(t*128+p) and 2*(t*128+p)+1 interleaved on
the free dim — exactly the (p, j) -> k = 2c+j pairing DoubleRow contracts,
with AP strides (j:1, m:2) expressed via bitcast + rearrange.  W transposes
happen SBUF->SBUF per 128-row o-block (no DRAM roundtrip for q).

Sharding: W/bias split along OUT across 8 cores (column parallel), x
replicated, per-core output [N, OUT/8] concatenated on the host along axis 1.
"""

import numpy as np

import concourse.bass as bass
import concourse.mybir as mybir
import concourse.tile as tile
from concourse import bacc
from concourse.bass_utils import run_bass_kernel_spmd

f32 = mybir.dt.float32
fp8 = mybir.dt.float8e4
u16 = mybir.dt.uint16
FP_MAGIC = 12582912.0  # 1.5 * 2**23: v + FP_MAGIC - FP_MAGIC == round-half-even(v)
QMAX = 7.0
R7 = float(np.float32(1.0) / np.float32(7.0))  # fl(1/7)
EPS = 1e-8

N_FULL, IN_FULL, OUT_FULL = 8192, 4096, 16384
CORES = 8


def declare_io(nc, n, in_, out_sh):
    xd = nc.dram_tensor("x", [n, in_], f32, kind="ExternalInput").ap()
    wd = nc.dram_tensor("weight", [out_sh, in_], f32, kind="ExternalInput").ap()
    bd = nc.dram_tensor("bias", [out_sh], f32, kind="ExternalInput").ap()
    outd = nc.dram_tensor("out", [n, out_sh], f32, kind="ExternalOutput").ap()
    return xd, wd, bd, outd


_REP_ID = [0]


def emit_program(tc, n, in_, out_sh, n_cores, io=None):
    nc = tc.nc
    if io is None:
        io = declare_io(nc, n, in_, out_sh)
    xd, wd, bd, outd = io
    rid = _REP_ID[0]
    _REP_ID[0] += 1
    add = mybir.AluOpType.add
    sub = mybir.AluOpType.subtract
    mult = mybir.AluOpType.mult
    mx = mybir.AluOpType.max
    copy_f = mybir.ActivationFunctionType.Copy
    ax_x = mybir.AxisListType.X
    DR = mybir.MatmulPerfMode.DoubleRow

    KC = in_ // 256          # u16 k-pair chunks (DoubleRow: 256 k per instr)
    nb = n // 128            # row blocks
    ot = min(512, out_sh)    # psum tile free dim
    not_ = out_sh // ot      # psum groups per row block (<=4 for 8 banks)
    assert not_ <= 4
    wrows = out_sh // 128
    xc = min(in_, 2048)      # x load chunk columns
    nxc = in_ // xc
    xkc = xc // 256          # u16 chunks per x load chunk

    cc_in = nc.dram_tensor(f"cc_in{rid}", [1], f32).ap()
    cc_out = nc.dram_tensor(f"cc_out{rid}", [1], f32, addr_space="Shared").ap()

    from contextlib import ExitStack

    with ExitStack() as ctx:
        const = ctx.enter_context(tc.tile_pool(name="const", bufs=1))
        xfp = ctx.enter_context(tc.tile_pool(name="xf", bufs=2))
        x8p = ctx.enter_context(tc.tile_pool(name="x8", bufs=2))
        xtp = ctx.enter_context(tc.tile_pool(name="xt", bufs=2))
        obp = ctx.enter_context(tc.tile_pool(name="ob", bufs=2))
        wtp = ctx.enter_context(tc.tile_pool(name="wt", bufs=1))

        # one padded slot holds all tiny scalar tiles
        nwt = wrows * (in_ // min(in_, 1024))
        misc = const.tile([128, 272 + nwt], f32, tag="misc")
        ones = misc[0:1, 0:128]
        scale_col = misc[:, 256:257]
        inv_col = misc[:, 257:258]
        amax = misc[0:1, 259:260]
        scale_s = misc[0:1, 260:261]
        part = misc[:, 272:272 + nwt]
        bias_rep = const.tile([128, out_sh], f32, tag="bias_rep")
        wtt = wtp.tile([128, KC, out_sh], u16, tag="wtt")

        nc.vector.memset(ones, 1.0)

        # ---- bias broadcast (independent of everything else) ----
        with tc.tile_pool(name="psprep", bufs=2, space="PSUM") as psprep:
            nc.sync.dma_start(bias_rep[0:1, :], bd)
            for j in range(out_sh // ot):
                pbias = psprep.tile([128, ot], f32, tag="brd", name="pbias")
                nc.tensor.matmul(pbias[:], ones,
                                 bias_rep[0:1, j * ot:(j + 1) * ot],
                                 start=True, stop=True)
                nc.scalar.copy(bias_rep[:, j * ot:(j + 1) * ot], pbias[:])

            # ---- absmax -> scale (own deep pool: DMA-rate streaming) ----
            awc = min(in_, 1024)
            anwc = in_ // awc
            with tc.tile_pool(name="wabs", bufs=6) as wabs:
                for t in range(wrows):
                    for c in range(anwc):
                        wt_ = wabs.tile([128, awc], f32, tag="aload")
                        nc.sync.dma_start(wt_[:], wd[t * 128:(t + 1) * 128,
                                                     c * awc:(c + 1) * awc])
                        i = t * anwc + c
                        nc.vector.tensor_reduce(part[:, i:i + 1], wt_[:],
                                                axis=ax_x, op=mx,
                                                apply_absolute_value=True)
            cmax = misc[:, 258:259]
            nc.vector.tensor_reduce(cmax, part[:], axis=ax_x, op=mx,
                                    apply_absolute_value=True)
            cmax_all = misc[:, 262:263]
            from concourse.bass import bass_isa
            nc.gpsimd.partition_all_reduce(cmax_all, cmax, 128,
                                           bass_isa.ReduceOp.max)
            nc.sync.dma_start(cc_in, cmax_all[0:1, 0:1])
            if n_cores > 1:
                nc.gpsimd.collective_compute(
                    "AllReduce", mx,
                    replica_groups=[list(range(n_cores))],
                    ins=[cc_in], outs=[cc_out])
            else:
                nc.sync.dma_start(cc_out, cc_in)
            nc.sync.dma_start(amax, cc_out)
            nc.vector.tensor_scalar(scale_s, amax, R7, None, mult)
            nc.vector.tensor_scalar(scale_s, scale_s, EPS, None, mx)
            pb = psprep.tile([128, 1], f32, tag="brd", name="pb")
            nc.tensor.matmul(pb[:], ones, scale_s, start=True, stop=True)
            nc.scalar.copy(scale_col, pb[:])
            nc.vector.reciprocal(inv_col, scale_col)

            # ---- quantize W -> fp8 q + u16 xbar transpose into resident
            # wtt (all SBUF; matmuls on psum group g can start once o-blocks
            # 4g..4g+3 have landed) ----
            with tc.tile_pool(name="wprep", bufs=2) as wp:
                for t in range(wrows):
                    wf = wp.tile([128, in_], f32, tag="wload")
                    nc.sync.dma_start(wf[:], wd[t * 128:(t + 1) * 128, :])
                    tq = wp.tile([128, in_], f32, tag="wmag")
                    # ACT: tq = w * inv + MAGIC  (rounds to integer in f32)
                    nc.scalar.activation(tq[:], wf[:], copy_f,
                                         bias=FP_MAGIC, scale=inv_col)
                    q8 = wp.tile([128, in_], fp8, tag="wq8")
                    # DVE: q8 = fp8(tq - MAGIC), integers in [-7,7] exact
                    nc.vector.tensor_scalar(q8[:], tq[:], FP_MAGIC, None, sub)
                    nc.sync.dma_start_transpose(
                        wtt[:, :, t * 128:(t + 1) * 128], q8[:].bitcast(u16))

        # ---- main loop ----
        with tc.tile_pool(name="psum", bufs=2, space="PSUM") as psp:
            for b in range(nb):
                xthi = xtp.tile([128, KC, 128], u16, tag="xthi")
                xtlo = xtp.tile([128, KC, 128], u16, tag="xtlo")
                for c2 in range(nxc):
                    xf = xfp.tile([128, xc], f32, tag="xf")
                    nc.sync.dma_start(xf[:], xd[b * 128:(b + 1) * 128,
                                                c2 * xc:(c2 + 1) * xc])
                    hi8 = x8p.tile([128, xc], fp8, tag="hi8")
                    nc.scalar.copy(hi8[:], xf[:])
                    lo8 = x8p.tile([128, xc], fp8, tag="lo8")
                    nc.vector.tensor_tensor(lo8[:], xf[:], hi8[:], sub)
                    nc.sync.dma_start_transpose(
                        xthi[:, c2 * xkc:(c2 + 1) * xkc, :],
                        hi8[:].bitcast(u16))
                    nc.sync.dma_start_transpose(
                        xtlo[:, c2 * xkc:(c2 + 1) * xkc, :],
                        lo8[:].bitcast(u16))
                # deinterleave to contiguous [128, KC, 2, 128] fp8: the DR
                # stationary must have (ksub, m) contiguous (ISA
                # s3_lw_dual_fp8_restrictions); the moving side may stay in
                # the u16-packed strided view.
                xdhi = xtp.tile([128, KC, 2, 128], fp8, tag="xdhi")
                xdlo = xtp.tile([128, KC, 2, 128], fp8, tag="xdlo")
                for xt_src, xt_dst in ((xthi, xdhi), (xtlo, xdlo)):
                    src = xt_src[:].bitcast(fp8).rearrange(
                        "p t (m j) -> p t j m", j=2)
                    nc.vector.tensor_scalar(xt_dst[:], src, 0.0, None, add)
                psums = [psp.tile([128, ot], f32, tag=f"mm{j}", name=f"ps{j}")
                         for j in range(not_)]
                for c in range(KC):
                    for pi, xt in enumerate((xdhi, xdlo)):
                        lhsT = xt[:, c, :, :]
                        for g in range(not_):
                            rhs = wtt[:, c, g * ot:(g + 1) * ot].bitcast(
                                fp8).rearrange("p (o j) -> p j o", j=2)
                            nc.tensor.matmul(psums[g][:], lhsT, rhs,
                                             start=(c == 0 and pi == 0),
                                             stop=(c == KC - 1 and pi == 1),
                                             perf_mode=DR)
                for g in range(not_):
                    ob = obp.tile([128, ot], f32, tag="ob")
                    co = g * ot
                    nc.vector.scalar_tensor_tensor(
                        ob[:], psums[g][:], scale_col, bias_rep[:, co:co + ot],
                        mult, add)
                    nc.sync.dma_start(outd[b * 128:(b + 1) * 128,
                                           co:co + ot], ob[:])


def build_nc(n=N_FULL, in_=IN_FULL, out_sh=OUT_FULL // CORES, n_cores=CORES,
             rep=1):
    nc = bacc.Bacc("TRN2", target_bir_lowering=False, debug=False,
                   enable_asserts=False, num_devices=n_cores)
    with tile.TileContext(nc) as tc:
        io = declare_io(nc, n, in_, out_sh)
        for _ in range(rep):
            emit_program(tc, n, in_, out_sh, n_cores, io=io)
    nc.compile()
    return nc


_NC_CACHE = {}


def _get_nc():
    key = (N_FULL, IN_FULL, OUT_FULL, CORES)
    if key not in _NC_CACHE:
        _NC_CACHE[key] = build_nc()
    return _NC_CACHE[key]


def kernel(x: np.ndarray, weight: np.ndarray, bias: np.ndarray) -> np.ndarray:
    assert x.shape == (N_FULL, IN_FULL)
    assert weight.shape == (OUT_FULL, IN_FULL)
    assert bias.shape == (OUT_FULL,)
    x = np.ascontiguousarray(x, dtype=np.float32)
    weight = np.ascontiguousarray(weight, dtype=np.float32)
    bias = np.ascontiguousarray(bias, dtype=np.float32)

    osh = OUT_FULL // CORES
    nc = _get_nc()
    in_maps = [
        {"x": x,
         "weight": weight[i * osh:(i + 1) * osh],
         "bias": bias[i * osh:(i + 1) * osh]}
        for i in range(CORES)
    ]
    res = run_bass_kernel_spmd(nc, in_maps, list(range(CORES))).results
    return np.concatenate([res[i]["out"] for i in range(CORES)], axis=1)
